# revision 1
# baseline (speedup 1.0000x reference)
"""GRU cell on 8 Trainium2 NeuronCores.

Reference computation (B=65536, D=256):
    z = sigmoid(x@Wz + h@Uz + bz)
    r = sigmoid(x@Wr + h@Ur + br)
    h_hat = tanh(x@Wh + (r*h)@Uh + bh)
    h_t = z*h + (1-z)*h_hat  ; returns (h_t, h_t)

Strategy: data-parallel over the batch dim (8 shards of 8192 rows).
The host pre-transposes each shard to [256, B_shard] so every on-chip
tensor lives in [hidden, batch] layout: the contraction dim of all six
GEMMs is then the SBUF partition dim with no on-chip transposes at all,
biases become per-partition ACT bias vectors, and the elementwise gate
math runs in the same layout the matmuls produce.  Matmul operands are
bitcast to float32r (full-rate PE mode for fp32 data).
"""

import os
import sys

for _p in ("/opt/trn_rl_repo", "/root/.axon_site/_ro/trn_rl_repo"):
    if os.path.isdir(_p) and _p not in sys.path:
        sys.path.append(_p)

import numpy as np

B = 65536
D = 256
N_CORES = 8
S = B // N_CORES  # batch rows per core
CH = 512  # batch columns per chunk (one PSUM bank of fp32)

_WNAMES = ("Wz", "Uz", "Wr", "Ur", "Wh", "Uh")
_BNAMES = ("bz", "br", "bh")


def build_nc(s=S, mm_dtype_name=None, ch=CH):
    """Build + compile the per-core Bass program for a shard of s rows."""
    import concourse.bass as bass
    import concourse.mybir as mybir
    import concourse.tile as tile
    from concourse import bacc

    f32 = mybir.dt.float32
    if mm_dtype_name is None:
        mm_dtype_name = os.environ.get("GRU_MM_DTYPE", "float32r")
    mm_dt = getattr(mybir.dt, mm_dtype_name)
    AF = mybir.ActivationFunctionType

    nc = bacc.Bacc("TRN2", target_bir_lowering=False)
    xT = nc.dram_tensor("xT", [D, s], f32, kind="ExternalInput")
    hT = nc.dram_tensor("hT", [D, s], f32, kind="ExternalInput")
    w_d = {n: nc.dram_tensor(n, [D, D], f32, kind="ExternalInput") for n in _WNAMES}
    b_d = {n: nc.dram_tensor(n, [D], f32, kind="ExternalInput") for n in _BNAMES}
    outT = nc.dram_tensor("outT", [D, s], f32, kind="ExternalOutput")

    nch = s // ch
    cast = mm_dt != f32
    # float32r is bit-identical to float32; allocate matmul operand tiles as
    # f32r and bitcast the fp32 views where engines need plain f32 semantics.
    f32r_mode = mm_dt == mybir.dt.float32r

    def md(ap):
        if ap.dtype == mm_dt:
            return ap
        return ap.bitcast(mm_dt) if cast else ap

    with tile.TileContext(nc) as tc:
        with (
            tc.tile_pool(name="const", bufs=1) as cpool,
            tc.tile_pool(name="inp", bufs=3) as ipool,
            tc.tile_pool(name="work", bufs=3) as wpool,
            tc.tile_pool(name="psum", bufs=1, space=bass.MemorySpace.PSUM) as ppool,
        ):
            # --- constants: weights [128, 256] x2 k-chunks each, biases [128, 2]
            w_sb = {}
            for n in _WNAMES:
                for k in range(2):
                    src = w_d[n][k * 128 : (k + 1) * 128, :]
                    if f32r_mode:
                        t = cpool.tile([128, D], mm_dt, tag=f"w_{n}_{k}")
                        nc.sync.dma_start(t[:], src.bitcast(mm_dt))
                    elif cast:
                        t0 = cpool.tile([128, D], f32, tag=f"wld_{n}_{k}")
                        nc.sync.dma_start(t0[:], src)
                        t = cpool.tile([128, D], mm_dt, tag=f"w_{n}_{k}")
                        nc.vector.tensor_copy(t[:], t0[:])
                    else:
                        t = cpool.tile([128, D], f32, tag=f"w_{n}_{k}")
                        nc.sync.dma_start(t[:], src)
                    w_sb[(n, k)] = t
            b_sb = {}
            for n in _BNAMES:
                t = cpool.tile([128, 2], f32, tag=f"b_{n}")
                nc.sync.dma_start(t[:], b_d[n].rearrange("(g p) -> p g", p=128))
                b_sb[n] = t

            def gate_psum(pool_tag, wn, un, rhs_w, rhs_u, g):
                """psum[{128},{ch}] = W[:,g].T @ rhs_w + U[:,g].T @ rhs_u."""
                p = ppool.tile([128, ch], f32, tag=pool_tag)
                gs = slice(g * 128, (g + 1) * 128)
                nc.tensor.matmul(p[:], md(w_sb[(wn, 0)][:, gs]), md(rhs_w[0][:]),
                                 start=True, stop=False)
                nc.tensor.matmul(p[:], md(w_sb[(wn, 1)][:, gs]), md(rhs_w[1][:]),
                                 start=False, stop=False)
                nc.tensor.matmul(p[:], md(w_sb[(un, 0)][:, gs]), md(rhs_u[0][:]),
                                 start=False, stop=False)
                nc.tensor.matmul(p[:], md(w_sb[(un, 1)][:, gs]), md(rhs_u[1][:]),
                                 start=False, stop=True)
                return p

            for c in range(nch):
                cols = slice(c * ch, (c + 1) * ch)
                # xt/ht: matmul-operand tiles; htf: f32 views of h for the
                # elementwise gate math.
                xt, ht, htf = [], [], []
                for k in range(2):
                    if f32r_mode:
                        tx = ipool.tile([128, ch], mm_dt, tag=f"x{k}")
                        nc.sync.dma_start(
                            tx[:], xT[k * 128 : (k + 1) * 128, cols].bitcast(mm_dt)
                        )
                        th = ipool.tile([128, ch], mm_dt, tag=f"h{k}")
                        nc.sync.dma_start(
                            th[:], hT[k * 128 : (k + 1) * 128, cols].bitcast(mm_dt)
                        )
                        xt.append(tx)
                        ht.append(th)
                        htf.append(th[:].bitcast(f32))
                    else:
                        tx = ipool.tile([128, ch], f32, tag=f"x{k}")
                        nc.sync.dma_start(tx[:], xT[k * 128 : (k + 1) * 128, cols])
                        th = ipool.tile([128, ch], f32, tag=f"h{k}")
                        nc.sync.dma_start(th[:], hT[k * 128 : (k + 1) * 128, cols])
                        htf.append(th[:])
                        if cast:
                            cx = ipool.tile([128, ch], mm_dt, tag=f"xc{k}")
                            nc.vector.tensor_copy(cx[:], tx[:])
                            chh = ipool.tile([128, ch], mm_dt, tag=f"hc{k}")
                            nc.vector.tensor_copy(chh[:], th[:])
                            xt.append(cx)
                            ht.append(chh)
                        else:
                            xt.append(tx)
                            ht.append(th)

                # reset gate -> r*h (needed before the candidate matmuls)
                rh = []
                for g in range(2):
                    pr = gate_psum(f"pr{g}", "Wr", "Ur", xt, ht, g)
                    rt = wpool.tile([128, ch], f32, tag=f"r{g}")
                    nc.scalar.activation(rt[:], pr[:], AF.Sigmoid,
                                         bias=b_sb["br"][:, g : g + 1])
                    t = wpool.tile([128, ch], mm_dt if cast else f32, tag=f"rh{g}")
                    nc.vector.tensor_mul(t[:], rt[:], htf[g])
                    rh.append(t)

                # update gate
                zt = []
                for g in range(2):
                    pz = gate_psum(f"pz{g}", "Wz", "Uz", xt, ht, g)
                    t = wpool.tile([128, ch], f32, tag=f"z{g}")
                    nc.scalar.activation(t[:], pz[:], AF.Sigmoid,
                                         bias=b_sb["bz"][:, g : g + 1])
                    zt.append(t)

                # candidate + combine + store
                for g in range(2):
                    ph = gate_psum(f"ph{g}", "Wh", "Uh", xt, rh, g)
                    hh = wpool.tile([128, ch], f32, tag=f"hh{g}")
                    nc.scalar.activation(hh[:], ph[:], AF.Tanh,
                                         bias=b_sb["bh"][:, g : g + 1])
                    d = wpool.tile([128, ch], f32, tag=f"d{g}")
                    nc.vector.tensor_sub(d[:], htf[g], hh[:])
                    m = wpool.tile([128, ch], f32, tag=f"m{g}")
                    nc.vector.tensor_mul(m[:], zt[g][:], d[:])
                    o = wpool.tile([128, ch], f32, tag=f"o{g}")
                    nc.vector.tensor_add(o[:], hh[:], m[:])
                    nc.sync.dma_start(outT[g * 128 : (g + 1) * 128, cols], o[:])

    nc.compile()
    return nc


_NC_CACHE = {}


def _get_nc():
    key = (S, os.environ.get("GRU_MM_DTYPE", "float32r"), CH)
    if key not in _NC_CACHE:
        _NC_CACHE[key] = build_nc(S, key[1], CH)
    return _NC_CACHE[key]


def _make_in_maps(inputs):
    f32 = np.float32
    x = np.asarray(inputs["x"], f32)
    h = np.asarray(inputs["h_t_1"], f32)
    consts = {n: np.ascontiguousarray(np.asarray(inputs[n], f32)) for n in _WNAMES}
    consts.update(
        {n: np.ascontiguousarray(np.asarray(inputs[n], f32)) for n in _BNAMES}
    )
    in_maps = []
    for c in range(N_CORES):
        sl = slice(c * S, (c + 1) * S)
        m = {
            "xT": np.ascontiguousarray(x[sl].T),
            "hT": np.ascontiguousarray(h[sl].T),
        }
        m.update(consts)
        in_maps.append(m)
    return in_maps


def run(inputs, trace=False):
    """Run on hardware; returns (h_t ndarray, BassKernelResults)."""
    from concourse.bass_utils import run_bass_kernel_spmd

    nc = _get_nc()
    in_maps = _make_in_maps(inputs)
    res = run_bass_kernel_spmd(nc, in_maps, list(range(N_CORES)), trace=trace)
    out = np.empty((B, D), np.float32)
    for c in range(N_CORES):
        out[c * S : (c + 1) * S] = res.results[c]["outT"].T
    return out, res


def kernel(**inputs):
    out, _ = run(inputs, trace=False)
    return (out, out)



# revision 6
# speedup vs baseline: 1.0446x; 1.0446x over previous
"""GRU cell on 8 Trainium2 NeuronCores.

Reference computation (B=65536, D=256):
    z = sigmoid(x@Wz + h@Uz + bz)
    r = sigmoid(x@Wr + h@Ur + br)
    h_hat = tanh(x@Wh + (r*h)@Uh + bh)
    h_t = z*h + (1-z)*h_hat  ; returns (h_t, h_t)

Strategy: data-parallel over the batch dim (8 shards of 8192 rows).
The host pre-transposes each shard to [256, B_shard] so every on-chip
tensor lives in [hidden, batch] layout: the contraction dim of all six
GEMMs is the SBUF partition dim with no on-chip transposes, biases are
per-partition ACT bias vectors, and the elementwise gate math runs in
the layout the matmuls produce.  Matmul operands are bitcast to
float32r (full-rate PE mode for fp32 data at moving-dim 512).

v2 changes vs the first working version (125.2us):
  * weights packed host-side into one [128,12,256] tensor, biases into
    [128,6] -> 4 const DMA instructions instead of 15 (each
    DMA_DIRECT2D costs ~0.6us of sync-queue issue time).
  * input loads merge both k-halves into one instruction and cover
    1024 batch columns per block; the first block is split 512+512 so
    the first matmul can start as early as possible.
  * DMA issue order puts x/h block0 + r-gate weights first.
  * output stores go through gpsimd (SWDGE) so they never queue behind
    input loads on the single sync HWDGE ring (was: stores stuck ~7us).
  * ~72 dependency-free bf16 warmup matmuls issued during the DMA
    prologue flip the PE HAM clock-gate to 2.4GHz before the real
    matmul stream begins (was: first ~3.4us of matmuls at 1.2GHz).
"""

import os
import sys

for _p in ("/opt/trn_rl_repo", "/root/.axon_site/_ro/trn_rl_repo"):
    if os.path.isdir(_p) and _p not in sys.path:
        sys.path.append(_p)

import numpy as np

B = 65536
D = 256
N_CORES = 8
S = B // N_CORES  # batch rows per core
CH = 512  # batch columns per compute chunk (one PSUM bank of fp32)
BLK = 1024  # batch columns per input-load block

# order of the 12 [128,256] weight slabs in the packed weight tensor
_WORDER = (("Wr", 0), ("Wr", 1), ("Ur", 0), ("Ur", 1),
           ("Wz", 0), ("Wz", 1), ("Uz", 0), ("Uz", 1),
           ("Wh", 0), ("Wh", 1), ("Uh", 0), ("Uh", 1))
_BORDER = ("br", "bz", "bh")  # bias col = 2*gate_idx + g


def build_nc(s=S, warm_mms=None):
    """Build + compile the per-core Bass program for a shard of s rows."""
    import concourse.bass as bass
    import concourse.mybir as mybir
    import concourse.tile as tile
    from concourse import bacc

    f32 = mybir.dt.float32
    f32r = mybir.dt.float32r
    bf16 = mybir.dt.bfloat16
    AF = mybir.ActivationFunctionType
    if warm_mms is None:
        warm_mms = int(os.environ.get("GRU_WARM", "72"))
    store_eng_name = os.environ.get("GRU_STORE", "gpsimd")

    nc = bacc.Bacc("TRN2", target_bir_lowering=False)
    xT = nc.dram_tensor("xT", [D, s], f32, kind="ExternalInput")
    hT = nc.dram_tensor("hT", [D, s], f32, kind="ExternalInput")
    wp_d = nc.dram_tensor("wpack", [128, 12, 256], f32, kind="ExternalInput")
    bp_d = nc.dram_tensor("bpack", [128, 6], f32, kind="ExternalInput")
    outT = nc.dram_tensor("outT", [D, s], f32, kind="ExternalOutput")

    xr = xT.rearrange("(k p) m -> p k m", p=128)  # [128, 2, s] dram view
    hr = hT.rearrange("(k p) m -> p k m", p=128)
    outr = outT.rearrange("(g p) m -> p g m", p=128)

    nblk = s // BLK
    store_eng = getattr(nc, store_eng_name)

    with tile.TileContext(nc) as tc:
        with (
            tc.tile_pool(name="const", bufs=1) as cpool,
            tc.tile_pool(name="inp", bufs=3) as ipool,
            tc.tile_pool(name="work", bufs=3) as wpool,
            tc.tile_pool(name="psum", bufs=1, space=bass.MemorySpace.PSUM) as ppool,
        ):
            # --- PE warmup: dependency-free bf16 matmuls flip the HAM
            # clock gate to 2.4GHz while the DMA prologue runs.
            pwarm = ppool.tile([128, CH], f32, tag="pwarm")
            if warm_mms:
                warm = cpool.tile([128, 128], bf16, tag="warm")
                nc.vector.memset(warm[:], 0.0)
                for _ in range(warm_mms):
                    nc.tensor.matmul(pwarm[:, 0:64], warm[:, 0:128],
                                     warm[:, 0:64], start=True, stop=True)

            # --- constants + first input block, issue-ordered for fast start
            # (tiles feeding the PE are allocated as f32r: the BIR verifier
            # requires fp32r-matmult inputs to be f32r-typed at the producer)
            wp_sb = cpool.tile([128, 12, 256], f32r, tag="wpack")
            b_sb = cpool.tile([128, 6], f32, tag="bpack")
            xb0 = ipool.tile([128, 2, BLK], f32r, tag="x")
            hb0 = ipool.tile([128, 2, BLK], f32r, tag="h")
            wp_r = wp_d.bitcast(f32r)
            nc.sync.dma_start(xb0[:, :, 0:CH], xr[:, :, 0:CH].bitcast(f32r))
            nc.sync.dma_start(hb0[:, :, 0:CH], hr[:, :, 0:CH].bitcast(f32r))
            nc.sync.dma_start(wp_sb[:, 0:4, :], wp_r[:, 0:4, :])  # Wr,Ur
            nc.sync.dma_start(b_sb[:], bp_d[:])
            nc.sync.dma_start(xb0[:, :, CH:BLK], xr[:, :, CH:BLK].bitcast(f32r))
            nc.sync.dma_start(hb0[:, :, CH:BLK], hr[:, :, CH:BLK].bitcast(f32r))
            nc.sync.dma_start(wp_sb[:, 4:8, :], wp_r[:, 4:8, :])  # Wz,Uz
            nc.sync.dma_start(wp_sb[:, 8:12, :], wp_r[:, 8:12, :])  # Wh,Uh

            def wsl(j, g):
                """Stationary [128,128] slab: weight j, output half g."""
                return wp_sb[:, j, g * 128:(g + 1) * 128]

            def gate_psum(tag, jw, ju, rhs_w, rhs_u, g):
                """psum[{128},{CH}] = W[:,g].T @ rhs_w + U[:,g].T @ rhs_u."""
                p = ppool.tile([128, CH], f32, tag=tag)
                nc.tensor.matmul(p[:], wsl(jw, g), rhs_w(0),
                                 start=True, stop=False)
                nc.tensor.matmul(p[:], wsl(jw + 1, g), rhs_w(1),
                                 start=False, stop=False)
                nc.tensor.matmul(p[:], wsl(ju, g), rhs_u(0),
                                 start=False, stop=False)
                nc.tensor.matmul(p[:], wsl(ju + 1, g), rhs_u(1),
                                 start=False, stop=True)
                return p

            for blk in range(nblk):
                if blk == 0:
                    xb, hb = xb0, hb0
                else:
                    bsl = slice(blk * BLK, (blk + 1) * BLK)
                    xb = ipool.tile([128, 2, BLK], f32r, tag="x")
                    nc.sync.dma_start(xb[:], xr[:, :, bsl].bitcast(f32r))
                    hb = ipool.tile([128, 2, BLK], f32r, tag="h")
                    nc.sync.dma_start(hb[:], hr[:, :, bsl].bitcast(f32r))

                for cc in range(BLK // CH):
                    csl = slice(cc * CH, (cc + 1) * CH)

                    def xk(k):
                        return xb[:, k, csl]

                    def hk(k):
                        return hb[:, k, csl]

                    def hf(k):
                        return hb[:, k, csl].bitcast(f32)

                    # reset gate -> r*h (needed before candidate matmuls)
                    rh = []
                    for g in range(2):
                        pr = gate_psum(f"pr{g}", 0, 2, xk, hk, g)
                        rt = wpool.tile([128, CH], f32, tag=f"r{g}")
                        nc.scalar.activation(rt[:], pr[:], AF.Sigmoid,
                                             bias=b_sb[:, g:g + 1])
                        t = wpool.tile([128, CH], f32r, tag=f"rh{g}")
                        nc.vector.tensor_mul(t[:], rt[:], hf(g))
                        rh.append(t)

                    # update gate
                    zt = []
                    for g in range(2):
                        pz = gate_psum(f"pz{g}", 4, 6, xk, hk, g)
                        t = wpool.tile([128, CH], f32, tag=f"z{g}")
                        nc.scalar.activation(t[:], pz[:], AF.Sigmoid,
                                             bias=b_sb[:, 2 + g:3 + g])
                        zt.append(t)

                    # candidate + combine + store
                    o = wpool.tile([128, 2, CH], f32, tag="o")
                    for g in range(2):
                        ph = gate_psum(f"ph{g}", 8, 10, xk,
                                       lambda k: rh[k][:], g)
                        hh = wpool.tile([128, CH], f32, tag=f"hh{g}")
                        nc.scalar.activation(hh[:], ph[:], AF.Tanh,
                                             bias=b_sb[:, 4 + g:5 + g])
                        dt_ = wpool.tile([128, CH], f32, tag=f"d{g}")
                        nc.vector.tensor_sub(dt_[:], hf(g), hh[:])
                        mt = wpool.tile([128, CH], f32, tag=f"m{g}")
                        nc.vector.tensor_mul(mt[:], zt[g][:], dt_[:])
                        nc.vector.tensor_add(o[:, g, :], hh[:], mt[:])
                    cols = slice(blk * BLK + cc * CH, blk * BLK + (cc + 1) * CH)
                    store_eng.dma_start(outr[:, :, cols], o[:])

    nc.compile()
    return nc


_NC_CACHE = {}


def _get_nc():
    key = (S, os.environ.get("GRU_WARM", "72"), os.environ.get("GRU_STORE", "gpsimd"))
    if key not in _NC_CACHE:
        _NC_CACHE[key] = build_nc(S)
    return _NC_CACHE[key]


def _make_in_maps(inputs):
    f32 = np.float32
    x = np.asarray(inputs["x"], f32)
    h = np.asarray(inputs["h_t_1"], f32)
    wpack = np.empty((128, 12, 256), f32)
    for j, (name, k) in enumerate(_WORDER):
        wpack[:, j, :] = np.asarray(inputs[name], f32)[k * 128:(k + 1) * 128, :]
    bpack = np.empty((128, 6), f32)
    for i, name in enumerate(_BORDER):
        b = np.asarray(inputs[name], f32)
        for g in range(2):
            bpack[:, 2 * i + g] = b[g * 128:(g + 1) * 128]
    consts = {"wpack": np.ascontiguousarray(wpack),
              "bpack": np.ascontiguousarray(bpack)}
    in_maps = []
    for c in range(N_CORES):
        sl = slice(c * S, (c + 1) * S)
        m = {
            "xT": np.ascontiguousarray(x[sl].T),
            "hT": np.ascontiguousarray(h[sl].T),
        }
        m.update(consts)
        in_maps.append(m)
    return in_maps


def run(inputs, trace=False):
    """Run on hardware; returns (h_t ndarray, BassKernelResults)."""
    from concourse.bass_utils import run_bass_kernel_spmd

    nc = _get_nc()
    in_maps = _make_in_maps(inputs)
    res = run_bass_kernel_spmd(nc, in_maps, list(range(N_CORES)), trace=trace)
    out = np.empty((B, D), np.float32)
    for c in range(N_CORES):
        out[c * S : (c + 1) * S] = res.results[c]["outT"].T
    return out, res


def kernel(**inputs):
    out, _ = run(inputs, trace=False)
    return (out, out)
